# revision 39
# baseline (speedup 1.0000x reference)
"""Trainium2 Bass kernel for a single attention head.

Problem: X[4,4096,1024], Wq/Wk/Wv[1024,128] ->
  softmax((X@Wq)(X@Wk)^T / sqrt(1024)) @ (X@Wv)   -> [4,4096,128]

Sharding: 8 cores = 4 batches x 2 query-halves. Each core receives the full
X of its batch (rolled so its query half is rows [0:2048)), computes K/V for
all 4096 keys and flash-style attention for its 2048 queries.

Pipeline (all matmuls bf16 inputs, fp32 PSUM accumulation):
  - X^T is pre-laid-out and rounded to bf16 on the host (pure relayout),
    so the device does plain chunked DMA loads of X^T -- no casting DMA
    (which measured ~126GB/s, 2.8x slower than plain DMA) and no XBAR
    transposes of X. Weights are host-prepped to bf16 tiles the same way.
  - Projections K^T/V^T/Q^T per 512-token chunk with two PSUM banks
    interleaved (K/V pairs) so matmul drains overlap; each pair is
    spread 4 matmuls per attention k-step so production never stalls
    the PE/ACT attention pipeline. Production of chunks 1-7 is
    interleaved into the first attention q-chunk.
  - Transposed flash attention, software-pipelined: S^T(kt+1) is issued
    to the PE before O^T(kt) so the PE has work during exp(kt) on ACT
    (which is the pacing engine: 64 x [128,1024] Exp activations).
  - exp outputs land in a 16-slice ring tile; the softmax denominator
    is accumulated by one contiguous 4-slice [128,4096] DVE add per 4
    k-tiles, and the cross-slice/partition sum is folded into a
    ones-matmul on the PE (fp32 PSUM), then reciprocal_approx_fast.
  - Epilogue: out_ps evacuated by a scalar-engine copy right after the
    last O matmul (frees PSUM for the next q-chunk); the l -> 1/l ->
    scale chain for q-chunk 0 is deferred into q-chunk 1's loop; O^T is
    DMA'd out transposed and un-transposed on the host (pure layout).
"""

import numpy as np

B, N, D, H = 4, 4096, 1024, 128
NCORES = 8
QSPLIT = 2  # cores per batch (query halves)
NQ = N // QSPLIT
SCALE = 1.0 / float(np.sqrt(np.float32(D)))
P = 128  # partitions
FB = 512  # matmul free-dim block (one fp32 PSUM bank)
CR = 512  # X rows per projection job
QC = 1024  # query chunk
DT = D // P   # 8 contraction tiles
NT = N // P   # 32 key tiles
NC = N // CR  # 8 projection jobs
XC = 8        # X DMA chunks
XCR = N // XC
KPC = CR // P  # 4 key tiles per chunk
PR = 16       # pT ring depth (slices)
GL = 4        # denominator group length (ring slices per DVE add)


def emit_attention(tc, XT, Ws, OT, n=N, d=D, nq=NQ):
    """Emit the single-core attention program into TileContext tc."""
    import concourse.mybir as mybir

    nc = tc.nc
    dt = mybir.dt
    f32, bf16 = dt.float32, dt.bfloat16
    AF = mybir.ActivationFunctionType
    AX = mybir.AxisListType
    ALU = mybir.AluOpType
    qc = QC
    NQC = nq // qc

    from contextlib import ExitStack

    with ExitStack() as ctx:
        cpool = ctx.enter_context(tc.tile_pool(name="const", bufs=1))
        big = ctx.enter_context(tc.tile_pool(name="big", bufs=1))
        vtp = ctx.enter_context(tc.tile_pool(name="vtp", bufs=2))
        gsp = ctx.enter_context(tc.tile_pool(name="gsp", bufs=2))
        epp = ctx.enter_context(tc.tile_pool(name="ep", bufs=2))
        # PSUM: p12 2x1 + stp 2x2 + accp 1x2 = 8 banks
        p12 = ctx.enter_context(tc.tile_pool(name="p12", bufs=2, space="PSUM"))
        stp = ctx.enter_context(tc.tile_pool(name="stps", bufs=2, space="PSUM"))
        accp = ctx.enter_context(tc.tile_pool(name="accps", bufs=1, space="PSUM"))

        ones_sq = cpool.tile([P, P], bf16)
        nc.vector.memset(ones_sq[:], 1.0)



        w_sb = {}

        def load_w(name):
            t = cpool.tile([P, DT * H], bf16, tag=name, name=f"w_{name}")
            nc.sync.dma_start(
                t[:].rearrange("p (t h) -> p t h", t=DT), Ws[name])
            w_sb[name] = t

        # X^T: xt[p, c, t, nb] = X^T[t*128+p, c*1024+nb] (DMA-chunk major)
        xt = big.tile([P, XC * DT * XCR], bf16)
        xt4 = xt[:].rearrange("p (c t nb) -> p c t nb", c=XC, t=DT)

        def xt_job(hc, t):
            """[128, 512] X^T slice for projection job hc, d-tile t."""
            c = hc * CR // XCR
            o = (hc * CR) % XCR
            return xt4[:, c, t, o:o + CR]
        kT = big.tile([P, n], bf16)          # K^T[h, keys]
        qT = big.tile([P, nq], bf16)         # Q^T[h, q]
        v_sb = big.tile([P, NT * H], bf16)   # V[k%128, kt*H + h]
        v_sb3 = v_sb[:].rearrange("p (kt h) -> p kt h", h=H)
        # exp ring: pT3[:, r, :] = P^T slice for k-tile with kt % PR == r
        pT_all = big.tile([P, PR * qc], bf16)
        pT3 = pT_all[:].rearrange("p (r q) -> p r q", r=PR)

        def produce_data(c):
            nc.sync.dma_start(xt4[:, c], XT[c])

        def proj_pair_stages(jobs, on_scalar=False):
            """Return 4 closures, each emitting 2 t-steps of the pair's
            interleaved matmuls; the last also emits copies/transposes."""
            state = {}

            def stage(si):
                def run():
                    if si == 0:
                        state['tiles'] = [
                            p12.tile([P, CR], f32, tag="pps",
                                     name=f"ps_{w}{c}")
                            for w, c in jobs]
                    for t in range(si * 2, si * 2 + 2):
                        for (wname, c), ps in zip(jobs, state['tiles']):
                            nc.tensor.matmul(
                                ps[:],
                                w_sb[wname][:, t * H:(t + 1) * H],
                                xt_job(c, t),
                                start=(t == 0),
                                stop=(t == DT - 1),
                            )
                    if si == 3:
                        for (wname, c), ps in zip(jobs, state['tiles']):
                            cp = (nc.scalar.copy if on_scalar
                                  else nc.vector.tensor_copy)
                            if wname == "wv":
                                vt = vtp.tile([P, CR], bf16, tag="vt",
                                              name=f"vt{c}")
                                cp(vt[:], ps[:])
                                nc.sync.dma_start_transpose(
                                    v_sb3[:, c * KPC:(c + 1) * KPC], vt[:])
                            else:
                                dst = kT if wname == "wk" else qT
                                cp(dst[:, c * CR:(c + 1) * CR], ps[:])
                return run
            return [stage(i) for i in range(4)]

        def proj_pair(jobs, on_scalar=False):
            for s in proj_pair_stages(jobs, on_scalar):
                s()

        # ---- Phase 1: X chunks 0-2 + K/V of tokens 0-511, Q of 0-1023
        produce_data(0)
        load_w("wk")
        load_w("wv")
        produce_data(1)
        load_w("wq")
        produce_data(2)
        proj_pair((("wk", 0), ("wv", 0)), on_scalar=True)
        proj_pair((("wq", 0), ("wq", 1)), on_scalar=True)

        def emit_S(q0, kt):
            st = stp.tile([P, qc], f32, tag="st", name=f"st{q0}_{kt}")
            for j in range(0, qc, FB):
                nc.tensor.matmul(
                    st[:, j:j + FB],
                    kT[:, kt * P:(kt + 1) * P],
                    qT[:, q0 + j:q0 + j + FB],
                    start=True, stop=True,
                )
            return st

        # deferred epilogue state from the previous q-chunk
        pending = {}

        def finish_epilogue():
            if not pending:
                return
            gs, ob, q0p = pending.pop('gs'), pending.pop('ob'), \
                pending.pop('q0')
            l_a = p12.tile([P, FB], f32, tag="pps", name=f"la{q0p}")
            l_b = p12.tile([P, FB], f32, tag="pps", name=f"lb{q0p}")
            ng = len(gs)
            r_sb = epp.tile([P, qc], f32, tag="rsb", name=f"rsb{q0p}")
            o_sb = epp.tile([P, qc], f32, tag="osb", name=f"osb{q0p}")
            for g, gt in enumerate(gs):
                nc.tensor.matmul(l_a[:], ones_sq[:], gt[:, 0:FB],
                                 start=(g == 0), stop=(g == ng - 1))
            nc.vector.reciprocal_approx_fast(r_sb[:, 0:FB], l_a[:])
            for g, gt in enumerate(gs):
                nc.tensor.matmul(l_b[:], ones_sq[:], gt[:, FB:qc],
                                 start=(g == 0), stop=(g == ng - 1))
            nc.gpsimd.tensor_mul(o_sb[:, 0:FB], ob[:, 0:FB], r_sb[:, 0:FB])
            nc.sync.dma_start(OT[:, q0p:q0p + FB], o_sb[:, 0:FB])
            nc.vector.reciprocal_approx_fast(r_sb[:, FB:qc], l_b[:])
            nc.vector.tensor_mul(o_sb[:, FB:qc], ob[:, FB:qc], r_sb[:, FB:qc])
            nc.sync.dma_start(OT[:, q0p + FB:q0p + qc], o_sb[:, FB:qc])

        for qi in range(NQC):
            q0 = qi * qc
            actions = {}
            if qi == 0:
                for i, c in enumerate(range(3, XC)):
                    actions.setdefault(i, []).append((produce_data, (c,)))
                pjobs = [(("wk", c), ("wv", c)) for c in range(1, NC)]
                pjobs.append((("wq", 2), ("wq", 3)))
                # K1/V1 compressed into the first two slots (needed by S(4))
                s10, s11, s12, s13 = proj_pair_stages(pjobs[0])
                actions.setdefault(0, []).extend([(s10, ()), (s11, ())])
                actions.setdefault(1, []).extend([(s12, ()), (s13, ())])
                at = 2
                for jobs in pjobs[1:]:
                    for s in proj_pair_stages(jobs):
                        actions.setdefault(at, []).append((s, ()))
                        at += 1
            else:
                actions.setdefault(5, []).append((finish_epilogue, ()))

            out_ps = accp.tile([P, qc], f32, tag="out", name=f"out{qi}")
            st_tiles = {0: emit_S(q0, 0)}
            # denominator accumulator: [p, 4, qc] bf16, four interleaved
            # partial sums combined by the epilogue ones-matmul
            acc4 = gsp.tile([P, GL * qc], bf16, tag="a4", name=f"a4_{qi}")
            acc43 = acc4[:].rearrange("p (i q) -> p i q", i=GL)
            for kt in range(NT):
                for fn, arg in actions.get(kt, ()):
                    fn(*arg)
                if kt + 1 < NT:
                    st_tiles[kt + 1] = emit_S(q0, kt + 1)
                # exp on ACT into the ring
                nc.scalar.activation(
                    pT3[:, kt % PR, :], st_tiles.pop(kt)[:],
                    AF.Exp, scale=SCALE)
                # O^T accumulation for the PREVIOUS kt (software pipeline)
                if kt > 0:
                    for j in range(0, qc, FB):
                        nc.tensor.matmul(
                            out_ps[:, j:j + FB],
                            v_sb3[:, kt - 1, :],
                            pT3[:, (kt - 1) % PR, j:j + FB],
                            start=(kt - 1 == 0), stop=False,
                        )
                # denominator: one contiguous 4-slice DVE add per 4 k-tiles;
                # the final GL slices (kt 28-31) skip the accumulator and go
                # straight into the epilogue ones-matmul, so the post-exp(31)
                # chain is just one matmul -> reciprocal -> scale -> DMA.
                if kt < NT - GL and kt % GL == GL - 1:
                    r0 = (kt - (GL - 1)) % PR
                    grp = pT_all[:, r0 * qc:(r0 + GL) * qc]
                    if kt == GL - 1:
                        nc.vector.tensor_copy(acc4[:], grp)
                    else:
                        nc.vector.tensor_add(acc4[:], acc4[:], grp)
            # last O^T tile
            for j in range(0, qc, FB):
                nc.tensor.matmul(
                    out_ps[:, j:j + FB],
                    v_sb3[:, NT - 1, :],
                    pT3[:, (NT - 1) % PR, j:j + FB],
                    start=False, stop=True,
                )
            # evacuate out_ps immediately (frees PSUM for next q-chunk)
            ob = epp.tile([P, qc], f32, tag="ob", name=f"ob{qi}")
            nc.scalar.copy(ob[:], out_ps[:])
            # l terms: 4 accumulator quarters + the last 4 raw ring slices
            terms = [acc43[:, i, :] for i in range(GL)]
            terms += [pT3[:, (NT - GL + i) % PR, :] for i in range(GL)]
            pending.update(gs=terms, ob=ob, q0=q0)

        finish_epilogue()


def build_bass(n=N, d=D, nq=NQ):
    import concourse.mybir as mybir
    from concourse import bacc
    from concourse.tile import TileContext

    dt = mybir.dt
    nc = bacc.Bacc("TRN2", target_bir_lowering=False, debug=False)
    XT = nc.dram_tensor(
        "XT", [XC, P, DT, XCR], dt.bfloat16, kind="ExternalInput").ap()
    Ws = {}
    for name in ("wq", "wk", "wv"):
        Ws[name] = nc.dram_tensor(
            name.upper(), [P, DT, H], dt.bfloat16, kind="ExternalInput").ap()
    OT = nc.dram_tensor("OT", [H, nq], dt.float32, kind="ExternalOutput").ap()

    with TileContext(nc) as tc:
        emit_attention(tc, XT, Ws, OT, n=n, d=d, nq=nq)
    nc.compile()  # bacc passes: split multi-waits into EVSEM chains, etc.
    return nc


_CACHED = {}


def _get_nc():
    if "nc" not in _CACHED:
        _CACHED["nc"] = build_bass()
    return _CACHED["nc"]


def _prep_w(w):
    import ml_dtypes
    # [D, H] f32 -> [128, DT, H] bf16 with w_t[p, t, h] = W[t*128+p, h]
    return np.ascontiguousarray(
        w.reshape(DT, P, H).transpose(1, 0, 2)).astype(ml_dtypes.bfloat16)


def _prep_xt(xb):
    import ml_dtypes
    # [N, D] f32 -> [XC, 128, DT, XCR] bf16:
    # XT[c, p, t, nb] = X[c*XCR+nb, t*128+p]
    x4 = xb.reshape(XC, XCR, DT, P)          # [c, nb, t, p]
    return np.ascontiguousarray(
        x4.transpose(0, 3, 2, 1)).astype(ml_dtypes.bfloat16)


def kernel(X, Wq, Wk, Wv, trace=False):
    """Full-input entry point: X [4,4096,1024] f32 -> [4,4096,128] f32."""
    from concourse.bass_utils import run_bass_kernel_spmd

    X = np.ascontiguousarray(X, dtype=np.float32)
    wmap = {"WQ": _prep_w(np.asarray(Wq, dtype=np.float32)),
            "WK": _prep_w(np.asarray(Wk, dtype=np.float32)),
            "WV": _prep_w(np.asarray(Wv, dtype=np.float32))}

    nc = _get_nc()
    in_maps = []
    for core in range(NCORES):
        b, half = core // QSPLIT, core % QSPLIT
        xb = X[b]
        if half:
            # roll so this core's queries are rows [0:NQ); key set is unchanged
            xb = np.concatenate([xb[NQ:], xb[:NQ]], axis=0)
        in_maps.append({"XT": _prep_xt(xb), **wmap})

    res = run_bass_kernel_spmd(
        nc, in_maps, core_ids=list(range(NCORES)), trace=trace
    )
    out = np.empty((B, N, H), dtype=np.float32)
    for core in range(NCORES):
        b, half = core // QSPLIT, core % QSPLIT
        out[b, half * NQ:(half + 1) * NQ] = res.results[core]["OT"].T
    if trace:
        return out, res
    return out


# revision 40
# speedup vs baseline: 1.1162x; 1.1162x over previous
"""Trainium2 Bass kernel for a single attention head.

Problem: X[4,4096,1024], Wq/Wk/Wv[1024,128] ->
  softmax((X@Wq)(X@Wk)^T / sqrt(1024)) @ (X@Wv)   -> [4,4096,128]

Sharding: 8 cores = 4 batches x 2 query-halves. Each core receives the full
X of its batch (rolled so its query half is rows [0:2048)), computes K/V for
all 4096 keys and flash-style attention for its 2048 queries.

Pipeline (all matmuls bf16 inputs, fp32 PSUM accumulation):
  - X^T is pre-laid-out and rounded to bf16 on the host (pure relayout),
    so the device does plain chunked DMA loads of X^T -- no casting DMA
    (which measured ~126GB/s, 2.8x slower than plain DMA) and no XBAR
    transposes of X. Weights are host-prepped to bf16 tiles the same way.
  - Projections K^T/V^T/Q^T per 512-token chunk with two PSUM banks
    interleaved (K/V pairs) so matmul drains overlap; each pair is
    spread 4 matmuls per attention k-step so production never stalls
    the PE/ACT attention pipeline. Production of chunks 1-7 is
    interleaved into the first attention q-chunk.
  - Transposed flash attention, software-pipelined: S^T(kt+1) is issued
    to the PE before O^T(kt) so the PE has work during exp(kt) on ACT
    (which is the pacing engine: 64 x [128,1024] Exp activations).
  - exp outputs land in a 16-slice ring tile; the softmax denominator
    is accumulated by one contiguous 4-slice [128,4096] DVE add per 4
    k-tiles, and the cross-slice/partition sum is folded into a
    ones-matmul on the PE (fp32 PSUM), then reciprocal_approx_fast.
  - Epilogue: out_ps evacuated by a scalar-engine copy right after the
    last O matmul (frees PSUM for the next q-chunk); the l -> 1/l ->
    scale chain for q-chunk 0 is deferred into q-chunk 1's loop; O^T is
    DMA'd out transposed and un-transposed on the host (pure layout).
"""

import numpy as np

B, N, D, H = 4, 4096, 1024, 128
NCORES = 8
QSPLIT = 2  # cores per batch (query halves)
NQ = N // QSPLIT
SCALE = 1.0 / float(np.sqrt(np.float32(D)))
P = 128  # partitions
FB = 512  # matmul free-dim block (one fp32 PSUM bank)
CR = 512  # X rows per projection job
QC = 1024  # query chunk
DT = D // P   # 8 contraction tiles
NT = N // P   # 32 key tiles
NC = N // CR  # 8 projection jobs
XC = 8        # X DMA chunks
XCR = N // XC
KPC = CR // P  # 4 key tiles per chunk
PR = 16       # pT ring depth (slices)
GL = 4        # denominator group length (ring slices per DVE add)


def emit_attention(tc, XT, Ws, OT, n=N, d=D, nq=NQ):
    """Emit the single-core attention program into TileContext tc."""
    import concourse.mybir as mybir

    nc = tc.nc
    dt = mybir.dt
    f32, bf16 = dt.float32, dt.bfloat16
    AF = mybir.ActivationFunctionType
    AX = mybir.AxisListType
    ALU = mybir.AluOpType
    qc = QC
    NQC = nq // qc

    from contextlib import ExitStack

    with ExitStack() as ctx:
        cpool = ctx.enter_context(tc.tile_pool(name="const", bufs=1))
        big = ctx.enter_context(tc.tile_pool(name="big", bufs=1))
        vtp = ctx.enter_context(tc.tile_pool(name="vtp", bufs=2))
        gsp = ctx.enter_context(tc.tile_pool(name="gsp", bufs=2))
        epp = ctx.enter_context(tc.tile_pool(name="ep", bufs=2))
        # PSUM: p12 2x1 + stp 2x2 + accp 1x2 = 8 banks
        p12 = ctx.enter_context(tc.tile_pool(name="p12", bufs=2, space="PSUM"))
        stp = ctx.enter_context(tc.tile_pool(name="stps", bufs=2, space="PSUM"))
        accp = ctx.enter_context(tc.tile_pool(name="accps", bufs=1, space="PSUM"))

        ones_sq = cpool.tile([P, P], bf16)
        nc.vector.memset(ones_sq[:], 1.0)



        w_sb = {}

        def load_w(name):
            t = cpool.tile([P, DT * H], bf16, tag=name, name=f"w_{name}")
            nc.sync.dma_start(
                t[:].rearrange("p (t h) -> p t h", t=DT), Ws[name])
            w_sb[name] = t

        # X^T: xt[p, c, t, nb] = X^T[t*128+p, c*1024+nb] (DMA-chunk major)
        xt = big.tile([P, XC * DT * XCR], bf16)
        xt4 = xt[:].rearrange("p (c t nb) -> p c t nb", c=XC, t=DT)

        def xt_job(hc, t):
            """[128, 512] X^T slice for projection job hc, d-tile t."""
            c = hc * CR // XCR
            o = (hc * CR) % XCR
            return xt4[:, c, t, o:o + CR]
        kT = big.tile([P, n], bf16)          # K^T[h, keys]
        qT = big.tile([P, nq], bf16)         # Q^T[h, q]
        v_sb = big.tile([P, NT * H], bf16)   # V[k%128, kt*H + h]
        v_sb3 = v_sb[:].rearrange("p (kt h) -> p kt h", h=H)
        # exp ring: pT3[:, r, :] = P^T slice for k-tile with kt % PR == r
        pT_all = big.tile([P, PR * qc], bf16)
        pT3 = pT_all[:].rearrange("p (r q) -> p r q", r=PR)

        def produce_data(c):
            nc.sync.dma_start(xt4[:, c], XT[c])

        def proj_pair_stages(jobs, on_scalar=False):
            """Return 4 closures, each emitting 2 t-steps of the pair's
            interleaved matmuls; the last also emits copies/transposes."""
            state = {}

            def stage(si):
                def run():
                    if si == 0:
                        state['tiles'] = [
                            p12.tile([P, CR], f32, tag="pps",
                                     name=f"ps_{w}{c}")
                            for w, c in jobs]
                    for t in range(si * 2, si * 2 + 2):
                        for (wname, c), ps in zip(jobs, state['tiles']):
                            nc.tensor.matmul(
                                ps[:],
                                w_sb[wname][:, t * H:(t + 1) * H],
                                xt_job(c, t),
                                start=(t == 0),
                                stop=(t == DT - 1),
                            )
                    if si == 3:
                        for (wname, c), ps in zip(jobs, state['tiles']):
                            cp = (nc.scalar.copy if on_scalar
                                  else nc.vector.tensor_copy)
                            if wname == "wv":
                                vt = vtp.tile([P, CR], bf16, tag="vt",
                                              name=f"vt{c}")
                                cp(vt[:], ps[:])
                                nc.sync.dma_start_transpose(
                                    v_sb3[:, c * KPC:(c + 1) * KPC], vt[:])
                            else:
                                dst = kT if wname == "wk" else qT
                                cp(dst[:, c * CR:(c + 1) * CR], ps[:])
                return run
            return [stage(i) for i in range(4)]

        def proj_pair(jobs, on_scalar=False):
            for s in proj_pair_stages(jobs, on_scalar):
                s()

        # ---- Phase 1: X chunks 0-2 + K/V of tokens 0-511, Q of 0-1023.
        # The four first projections run 4-wide across 4 PSUM banks (the
        # attention st banks are still free here), so their fills/drains
        # overlap and no pair waits on the other's PSUM-freeing copies.
        produce_data(0)
        load_w("wk")
        load_w("wv")
        produce_data(1)
        load_w("wq")
        produce_data(2)
        qjobs = (("wk", 0), ("wv", 0), ("wq", 0), ("wq", 1))
        qtiles = [p12.tile([P, CR], f32, tag="pps", name=f"qp_{w}{c}")
                  for w, c in qjobs[:2]]
        qtiles += [stp.tile([P, CR], f32, tag="st", name=f"qp_{w}{c}")
                   for w, c in qjobs[2:]]
        for t in range(DT):
            for (wname, c), ps in zip(qjobs, qtiles):
                nc.tensor.matmul(
                    ps[:], w_sb[wname][:, t * H:(t + 1) * H], xt_job(c, t),
                    start=(t == 0), stop=(t == DT - 1))
        for (wname, c), ps in zip(qjobs, qtiles):
            if wname == "wv":
                vt = vtp.tile([P, CR], bf16, tag="vt", name=f"vt{c}")
                nc.scalar.copy(vt[:], ps[:])
                nc.sync.dma_start_transpose(
                    v_sb3[:, c * KPC:(c + 1) * KPC], vt[:])
            else:
                dst = kT if wname == "wk" else qT
                nc.scalar.copy(dst[:, c * CR:(c + 1) * CR], ps[:])

        def emit_S(q0, kt):
            st = stp.tile([P, qc], f32, tag="st", name=f"st{q0}_{kt}")
            for j in range(0, qc, FB):
                nc.tensor.matmul(
                    st[:, j:j + FB],
                    kT[:, kt * P:(kt + 1) * P],
                    qT[:, q0 + j:q0 + j + FB],
                    start=True, stop=True,
                )
            return st

        # deferred epilogue state from the previous q-chunk
        pending = {}

        def finish_epilogue():
            if not pending:
                return
            gs, ob, q0p = pending.pop('gs'), pending.pop('ob'), \
                pending.pop('q0')
            l_a = p12.tile([P, FB], f32, tag="pps", name=f"la{q0p}")
            l_b = p12.tile([P, FB], f32, tag="pps", name=f"lb{q0p}")
            ng = len(gs)
            r_sb = epp.tile([P, qc], f32, tag="rsb", name=f"rsb{q0p}")
            o_sb = epp.tile([P, qc], f32, tag="osb", name=f"osb{q0p}")
            for g, gt in enumerate(gs):
                nc.tensor.matmul(l_a[:], ones_sq[:], gt[:, 0:FB],
                                 start=(g == 0), stop=(g == ng - 1))
            nc.vector.reciprocal_approx_fast(r_sb[:, 0:FB], l_a[:])
            for g, gt in enumerate(gs):
                nc.tensor.matmul(l_b[:], ones_sq[:], gt[:, FB:qc],
                                 start=(g == 0), stop=(g == ng - 1))
            nc.gpsimd.tensor_mul(o_sb[:, 0:FB], ob[:, 0:FB], r_sb[:, 0:FB])
            nc.sync.dma_start(OT[:, q0p:q0p + FB], o_sb[:, 0:FB])
            nc.vector.reciprocal_approx_fast(r_sb[:, FB:qc], l_b[:])
            nc.vector.tensor_mul(o_sb[:, FB:qc], ob[:, FB:qc], r_sb[:, FB:qc])
            nc.sync.dma_start(OT[:, q0p + FB:q0p + qc], o_sb[:, FB:qc])

        for qi in range(NQC):
            q0 = qi * qc
            actions = {}
            if qi == 0:
                for i, c in enumerate(range(3, XC)):
                    actions.setdefault(i, []).append((produce_data, (c,)))
                pjobs = [(("wk", c), ("wv", c)) for c in range(1, NC)]
                pjobs.append((("wq", 2), ("wq", 3)))
                # K1/V1 compressed into the first two slots (needed by S(4))
                s10, s11, s12, s13 = proj_pair_stages(pjobs[0])
                actions.setdefault(0, []).extend([(s10, ()), (s11, ())])
                actions.setdefault(1, []).extend([(s12, ()), (s13, ())])
                at = 2
                for jobs in pjobs[1:]:
                    for s in proj_pair_stages(jobs):
                        actions.setdefault(at, []).append((s, ()))
                        at += 1
            else:
                actions.setdefault(5, []).append((finish_epilogue, ()))

            out_ps = accp.tile([P, qc], f32, tag="out", name=f"out{qi}")
            st_tiles = {0: emit_S(q0, 0)}
            # denominator accumulator: [p, 4, qc] bf16, four interleaved
            # partial sums combined by the epilogue ones-matmul
            acc4 = gsp.tile([P, GL * qc], bf16, tag="a4", name=f"a4_{qi}")
            acc43 = acc4[:].rearrange("p (i q) -> p i q", i=GL)
            for kt in range(NT):
                for fn, arg in actions.get(kt, ()):
                    fn(*arg)
                if kt + 1 < NT:
                    st_tiles[kt + 1] = emit_S(q0, kt + 1)
                # exp on ACT into the ring
                nc.scalar.activation(
                    pT3[:, kt % PR, :], st_tiles.pop(kt)[:],
                    AF.Exp, scale=SCALE)
                # O^T accumulation for the PREVIOUS kt (software pipeline)
                if kt > 0:
                    for j in range(0, qc, FB):
                        nc.tensor.matmul(
                            out_ps[:, j:j + FB],
                            v_sb3[:, kt - 1, :],
                            pT3[:, (kt - 1) % PR, j:j + FB],
                            start=(kt - 1 == 0), stop=False,
                        )
                # denominator: one contiguous 4-slice DVE add per 4 k-tiles;
                # the final GL slices (kt 28-31) skip the accumulator and go
                # straight into the epilogue ones-matmul, so the post-exp(31)
                # chain is just one matmul -> reciprocal -> scale -> DMA.
                if kt < NT - GL and kt % GL == GL - 1:
                    r0 = (kt - (GL - 1)) % PR
                    grp = pT_all[:, r0 * qc:(r0 + GL) * qc]
                    if kt == GL - 1:
                        nc.vector.tensor_copy(acc4[:], grp)
                    else:
                        nc.vector.tensor_add(acc4[:], acc4[:], grp)
            # last O^T tile
            for j in range(0, qc, FB):
                nc.tensor.matmul(
                    out_ps[:, j:j + FB],
                    v_sb3[:, NT - 1, :],
                    pT3[:, (NT - 1) % PR, j:j + FB],
                    start=False, stop=True,
                )
            # evacuate out_ps immediately (frees PSUM for next q-chunk)
            ob = epp.tile([P, qc], f32, tag="ob", name=f"ob{qi}")
            nc.scalar.copy(ob[:], out_ps[:])
            # l terms: 4 accumulator quarters + the last 4 raw ring slices
            terms = [acc43[:, i, :] for i in range(GL)]
            terms += [pT3[:, (NT - GL + i) % PR, :] for i in range(GL)]
            pending.update(gs=terms, ob=ob, q0=q0)

        finish_epilogue()


def build_bass(n=N, d=D, nq=NQ):
    import concourse.mybir as mybir
    from concourse import bacc
    from concourse.tile import TileContext

    dt = mybir.dt
    nc = bacc.Bacc("TRN2", target_bir_lowering=False, debug=False)
    XT = nc.dram_tensor(
        "XT", [XC, P, DT, XCR], dt.bfloat16, kind="ExternalInput").ap()
    Ws = {}
    for name in ("wq", "wk", "wv"):
        Ws[name] = nc.dram_tensor(
            name.upper(), [P, DT, H], dt.bfloat16, kind="ExternalInput").ap()
    OT = nc.dram_tensor("OT", [H, nq], dt.float32, kind="ExternalOutput").ap()

    with TileContext(nc) as tc:
        emit_attention(tc, XT, Ws, OT, n=n, d=d, nq=nq)
    nc.compile()  # bacc passes: split multi-waits into EVSEM chains, etc.
    return nc


_CACHED = {}


def _get_nc():
    if "nc" not in _CACHED:
        _CACHED["nc"] = build_bass()
    return _CACHED["nc"]


def _prep_w(w):
    import ml_dtypes
    # [D, H] f32 -> [128, DT, H] bf16 with w_t[p, t, h] = W[t*128+p, h]
    return np.ascontiguousarray(
        w.reshape(DT, P, H).transpose(1, 0, 2)).astype(ml_dtypes.bfloat16)


def _prep_xt(xb):
    import ml_dtypes
    # [N, D] f32 -> [XC, 128, DT, XCR] bf16:
    # XT[c, p, t, nb] = X[c*XCR+nb, t*128+p]
    x4 = xb.reshape(XC, XCR, DT, P)          # [c, nb, t, p]
    return np.ascontiguousarray(
        x4.transpose(0, 3, 2, 1)).astype(ml_dtypes.bfloat16)


def kernel(X, Wq, Wk, Wv, trace=False):
    """Full-input entry point: X [4,4096,1024] f32 -> [4,4096,128] f32."""
    from concourse.bass_utils import run_bass_kernel_spmd

    X = np.ascontiguousarray(X, dtype=np.float32)
    wmap = {"WQ": _prep_w(np.asarray(Wq, dtype=np.float32)),
            "WK": _prep_w(np.asarray(Wk, dtype=np.float32)),
            "WV": _prep_w(np.asarray(Wv, dtype=np.float32))}

    nc = _get_nc()
    in_maps = []
    for core in range(NCORES):
        b, half = core // QSPLIT, core % QSPLIT
        xb = X[b]
        if half:
            # roll so this core's queries are rows [0:NQ); key set is unchanged
            xb = np.concatenate([xb[NQ:], xb[:NQ]], axis=0)
        in_maps.append({"XT": _prep_xt(xb), **wmap})

    res = run_bass_kernel_spmd(
        nc, in_maps, core_ids=list(range(NCORES)), trace=trace
    )
    out = np.empty((B, N, H), dtype=np.float32)
    for core in range(NCORES):
        b, half = core // QSPLIT, core % QSPLIT
        out[b, half * NQ:(half + 1) * NQ] = res.results[core]["OT"].T
    if trace:
        return out, res
    return out


# revision 41
# speedup vs baseline: 1.1519x; 1.0319x over previous
"""Trainium2 Bass kernel for a single attention head.

Problem: X[4,4096,1024], Wq/Wk/Wv[1024,128] ->
  softmax((X@Wq)(X@Wk)^T / sqrt(1024)) @ (X@Wv)   -> [4,4096,128]

Sharding: 8 cores = 4 batches x 2 query-halves. Each core receives the full
X of its batch (rolled so its query half is rows [0:2048)), computes K/V for
all 4096 keys and flash-style attention for its 2048 queries.

Pipeline (all matmuls bf16 inputs, fp32 PSUM accumulation):
  - X^T is pre-laid-out and rounded to bf16 on the host (pure relayout),
    so the device does plain chunked DMA loads of X^T -- no casting DMA
    (which measured ~126GB/s, 2.8x slower than plain DMA) and no XBAR
    transposes of X. Weights are host-prepped to bf16 tiles the same way.
  - Projections K^T/V^T/Q^T per 512-token chunk with two PSUM banks
    interleaved (K/V pairs) so matmul drains overlap; each pair is
    spread 4 matmuls per attention k-step so production never stalls
    the PE/ACT attention pipeline. Production of chunks 1-7 is
    interleaved into the first attention q-chunk.
  - Transposed flash attention, software-pipelined: S^T(kt+1) is issued
    to the PE before O^T(kt) so the PE has work during exp(kt) on ACT
    (which is the pacing engine: 64 x [128,1024] Exp activations).
  - exp outputs land in a 16-slice ring tile; the softmax denominator
    is accumulated by one contiguous 4-slice [128,4096] DVE add per 4
    k-tiles, and the cross-slice/partition sum is folded into a
    ones-matmul on the PE (fp32 PSUM), then reciprocal_approx_fast.
  - Epilogue: out_ps evacuated by a scalar-engine copy right after the
    last O matmul (frees PSUM for the next q-chunk); the l -> 1/l ->
    scale chain for q-chunk 0 is deferred into q-chunk 1's loop; O^T is
    DMA'd out transposed and un-transposed on the host (pure layout).
"""

import numpy as np

B, N, D, H = 4, 4096, 1024, 128
NCORES = 8
QSPLIT = 2  # cores per batch (query halves)
NQ = N // QSPLIT
SCALE = 1.0 / float(np.sqrt(np.float32(D)))
P = 128  # partitions
FB = 512  # matmul free-dim block (one fp32 PSUM bank)
CR = 512  # X rows per projection job
QC = 1024  # query chunk
DT = D // P   # 8 contraction tiles
NT = N // P   # 32 key tiles
NC = N // CR  # 8 projection jobs
XC = 8        # X DMA chunks
XCR = N // XC
KPC = CR // P  # 4 key tiles per chunk
PR = 16       # pT ring depth (slices)
GL = 4        # denominator group length (ring slices per DVE add)


def emit_attention(tc, XT, Ws, OT, n=N, d=D, nq=NQ):
    """Emit the single-core attention program into TileContext tc."""
    import concourse.mybir as mybir

    nc = tc.nc
    dt = mybir.dt
    f32, bf16 = dt.float32, dt.bfloat16
    AF = mybir.ActivationFunctionType
    AX = mybir.AxisListType
    ALU = mybir.AluOpType
    qc = QC
    NQC = nq // qc

    from contextlib import ExitStack

    with ExitStack() as ctx:
        cpool = ctx.enter_context(tc.tile_pool(name="const", bufs=1))
        big = ctx.enter_context(tc.tile_pool(name="big", bufs=1))
        vtp = ctx.enter_context(tc.tile_pool(name="vtp", bufs=2))
        gsp = ctx.enter_context(tc.tile_pool(name="gsp", bufs=2))
        epp = ctx.enter_context(tc.tile_pool(name="ep", bufs=2))
        # PSUM: p12 2x1 + stp 2x2 + accp 1x2 = 8 banks
        p12 = ctx.enter_context(tc.tile_pool(name="p12", bufs=2, space="PSUM"))
        stp = ctx.enter_context(tc.tile_pool(name="stps", bufs=2, space="PSUM"))
        accp = ctx.enter_context(tc.tile_pool(name="accps", bufs=1, space="PSUM"))

        ones_sq = cpool.tile([P, P], bf16)
        nc.vector.memset(ones_sq[:], 1.0)



        w_sb = {}

        def load_w(name):
            t = cpool.tile([P, DT * H], bf16, tag=name, name=f"w_{name}")
            nc.sync.dma_start(
                t[:].rearrange("p (t h) -> p t h", t=DT), Ws[name])
            w_sb[name] = t

        # X^T: xt[p, c, t, nb] = X^T[t*128+p, c*1024+nb] (DMA-chunk major)
        xt = big.tile([P, XC * DT * XCR], bf16)
        xt4 = xt[:].rearrange("p (c t nb) -> p c t nb", c=XC, t=DT)

        def xt_job(hc, t):
            """[128, 512] X^T slice for projection job hc, d-tile t."""
            c = hc * CR // XCR
            o = (hc * CR) % XCR
            return xt4[:, c, t, o:o + CR]
        kT = big.tile([P, n], bf16)          # K^T[h, keys]
        qT = big.tile([P, nq], bf16)         # Q^T[h, q]
        v_sb = big.tile([P, NT * H], bf16)   # V[k%128, kt*H + h]
        v_sb3 = v_sb[:].rearrange("p (kt h) -> p kt h", h=H)
        # exp ring: pT3[:, r, :] = P^T slice for k-tile with kt % PR == r
        pT_all = big.tile([P, PR * qc], bf16)
        pT3 = pT_all[:].rearrange("p (r q) -> p r q", r=PR)

        def produce_data(c):
            nc.sync.dma_start(xt4[:, c], XT[c])

        def proj_pair_stages(jobs, on_scalar=False):
            """Return 4 closures, each emitting 2 t-steps of the pair's
            interleaved matmuls; the last also emits copies/transposes."""
            state = {}

            def stage(si):
                def run():
                    if si == 0:
                        state['tiles'] = [
                            p12.tile([P, CR], f32, tag="pps",
                                     name=f"ps_{w}{c}")
                            for w, c in jobs]
                    for t in range(si * 2, si * 2 + 2):
                        for (wname, c), ps in zip(jobs, state['tiles']):
                            nc.tensor.matmul(
                                ps[:],
                                w_sb[wname][:, t * H:(t + 1) * H],
                                xt_job(c, t),
                                start=(t == 0),
                                stop=(t == DT - 1),
                            )
                    if si == 3:
                        for (wname, c), ps in zip(jobs, state['tiles']):
                            cp = (nc.scalar.copy if on_scalar
                                  else nc.vector.tensor_copy)
                            if wname == "wv":
                                vt = vtp.tile([P, CR], bf16, tag="vt",
                                              name=f"vt{c}")
                                cp(vt[:], ps[:])
                                nc.sync.dma_start_transpose(
                                    v_sb3[:, c * KPC:(c + 1) * KPC], vt[:])
                            else:
                                dst = kT if wname == "wk" else qT
                                cp(dst[:, c * CR:(c + 1) * CR], ps[:])
                return run
            return [stage(i) for i in range(4)]

        def proj_pair(jobs, on_scalar=False):
            for s in proj_pair_stages(jobs, on_scalar):
                s()

        # ---- Phase 1: X chunks 0-2 + K/V of tokens 0-511, Q of 0-1023
        produce_data(0)
        load_w("wk")
        load_w("wv")
        produce_data(1)
        load_w("wq")
        produce_data(2)
        proj_pair((("wk", 0), ("wv", 0)), on_scalar=True)
        proj_pair((("wq", 0), ("wq", 1)), on_scalar=True)

        def emit_S(q0, kt):
            st = stp.tile([P, qc], f32, tag="st", name=f"st{q0}_{kt}")
            for j in range(0, qc, FB):
                nc.tensor.matmul(
                    st[:, j:j + FB],
                    kT[:, kt * P:(kt + 1) * P],
                    qT[:, q0 + j:q0 + j + FB],
                    start=True, stop=True,
                )
            return st

        # deferred epilogue state from the previous q-chunk
        pending = {}

        def finish_epilogue():
            if not pending:
                return
            gs, ob, q0p = pending.pop('gs'), pending.pop('ob'), \
                pending.pop('q0')
            l_a = p12.tile([P, FB], f32, tag="pps", name=f"la{q0p}")
            l_b = p12.tile([P, FB], f32, tag="pps", name=f"lb{q0p}")
            ng = len(gs)
            r_sb = epp.tile([P, qc], f32, tag="rsb", name=f"rsb{q0p}")
            o_sb = epp.tile([P, qc], f32, tag="osb", name=f"osb{q0p}")
            for g, gt in enumerate(gs):
                nc.tensor.matmul(l_a[:], ones_sq[:], gt[:, 0:FB],
                                 start=(g == 0), stop=(g == ng - 1))
            nc.vector.reciprocal_approx_fast(r_sb[:, 0:FB], l_a[:])
            for g, gt in enumerate(gs):
                nc.tensor.matmul(l_b[:], ones_sq[:], gt[:, FB:qc],
                                 start=(g == 0), stop=(g == ng - 1))
            nc.gpsimd.tensor_mul(o_sb[:, 0:FB], ob[:, 0:FB], r_sb[:, 0:FB])
            nc.sync.dma_start(OT[:, q0p:q0p + FB], o_sb[:, 0:FB])
            nc.vector.reciprocal_approx_fast(r_sb[:, FB:qc], l_b[:])
            nc.vector.tensor_mul(o_sb[:, FB:qc], ob[:, FB:qc], r_sb[:, FB:qc])
            nc.sync.dma_start(OT[:, q0p + FB:q0p + qc], o_sb[:, FB:qc])

        for qi in range(NQC):
            q0 = qi * qc
            actions = {}
            if qi == 0:
                for i, c in enumerate(range(3, XC)):
                    actions.setdefault(i, []).append((produce_data, (c,)))
                pjobs = [(("wk", c), ("wv", c)) for c in range(1, NC)]
                pjobs.append((("wq", 2), ("wq", 3)))
                # K1/V1 compressed into the first two slots (needed by S(4))
                s10, s11, s12, s13 = proj_pair_stages(pjobs[0])
                actions.setdefault(0, []).extend([(s10, ()), (s11, ())])
                actions.setdefault(1, []).extend([(s12, ()), (s13, ())])
                at = 2
                for jobs in pjobs[1:]:
                    for s in proj_pair_stages(jobs):
                        actions.setdefault(at, []).append((s, ()))
                        at += 1
            else:
                actions.setdefault(5, []).append((finish_epilogue, ()))

            out_ps = accp.tile([P, qc], f32, tag="out", name=f"out{qi}")
            st_tiles = {0: emit_S(q0, 0)}
            # denominator accumulator: [p, 4, qc] bf16, four interleaved
            # partial sums combined by the epilogue ones-matmul
            acc4 = gsp.tile([P, GL * qc], bf16, tag="a4", name=f"a4_{qi}")
            acc43 = acc4[:].rearrange("p (i q) -> p i q", i=GL)
            for kt in range(NT):
                for fn, arg in actions.get(kt, ()):
                    fn(*arg)
                if kt + 1 < NT:
                    st_tiles[kt + 1] = emit_S(q0, kt + 1)
                # exp on ACT into the ring
                nc.scalar.activation(
                    pT3[:, kt % PR, :], st_tiles.pop(kt)[:],
                    AF.Exp, scale=SCALE)
                # O^T accumulation for the PREVIOUS kt (software pipeline)
                if kt > 0:
                    for j in range(0, qc, FB):
                        nc.tensor.matmul(
                            out_ps[:, j:j + FB],
                            v_sb3[:, kt - 1, :],
                            pT3[:, (kt - 1) % PR, j:j + FB],
                            start=(kt - 1 == 0), stop=False,
                        )
                # denominator: one contiguous 4-slice DVE add per 4 k-tiles;
                # the final GL slices (kt 28-31) skip the accumulator and go
                # straight into the epilogue ones-matmul, so the post-exp(31)
                # chain is just one matmul -> reciprocal -> scale -> DMA.
                if kt < NT - GL and kt % GL == GL - 1:
                    r0 = (kt - (GL - 1)) % PR
                    grp = pT_all[:, r0 * qc:(r0 + GL) * qc]
                    if kt == GL - 1:
                        nc.vector.tensor_copy(acc4[:], grp)
                    else:
                        nc.vector.tensor_add(acc4[:], acc4[:], grp)
            # last O^T tile
            for j in range(0, qc, FB):
                nc.tensor.matmul(
                    out_ps[:, j:j + FB],
                    v_sb3[:, NT - 1, :],
                    pT3[:, (NT - 1) % PR, j:j + FB],
                    start=False, stop=True,
                )
            # evacuate out_ps immediately (frees PSUM for next q-chunk)
            ob = epp.tile([P, qc], f32, tag="ob", name=f"ob{qi}")
            nc.scalar.copy(ob[:], out_ps[:])
            # l terms: 4 accumulator quarters + the last 4 raw ring slices
            terms = [acc43[:, i, :] for i in range(GL)]
            terms += [pT3[:, (NT - GL + i) % PR, :] for i in range(GL)]
            pending.update(gs=terms, ob=ob, q0=q0)

        finish_epilogue()


def build_bass(n=N, d=D, nq=NQ):
    import concourse.mybir as mybir
    from concourse import bacc
    from concourse.tile import TileContext

    dt = mybir.dt
    nc = bacc.Bacc("TRN2", target_bir_lowering=False, debug=False)
    XT = nc.dram_tensor(
        "XT", [XC, P, DT, XCR], dt.bfloat16, kind="ExternalInput").ap()
    Ws = {}
    for name in ("wq", "wk", "wv"):
        Ws[name] = nc.dram_tensor(
            name.upper(), [P, DT, H], dt.bfloat16, kind="ExternalInput").ap()
    OT = nc.dram_tensor("OT", [H, nq], dt.float32, kind="ExternalOutput").ap()

    with TileContext(nc) as tc:
        emit_attention(tc, XT, Ws, OT, n=n, d=d, nq=nq)
    nc.compile()  # bacc passes: split multi-waits into EVSEM chains, etc.
    return nc


_CACHED = {}


def _get_nc():
    if "nc" not in _CACHED:
        _CACHED["nc"] = build_bass()
    return _CACHED["nc"]


def _prep_w(w):
    import ml_dtypes
    # [D, H] f32 -> [128, DT, H] bf16 with w_t[p, t, h] = W[t*128+p, h]
    return np.ascontiguousarray(
        w.reshape(DT, P, H).transpose(1, 0, 2)).astype(ml_dtypes.bfloat16)


def _prep_xt(xb):
    import ml_dtypes
    # [N, D] f32 -> [XC, 128, DT, XCR] bf16:
    # XT[c, p, t, nb] = X[c*XCR+nb, t*128+p]
    x4 = xb.reshape(XC, XCR, DT, P)          # [c, nb, t, p]
    return np.ascontiguousarray(
        x4.transpose(0, 3, 2, 1)).astype(ml_dtypes.bfloat16)


def kernel(X, Wq, Wk, Wv, trace=False):
    """Full-input entry point: X [4,4096,1024] f32 -> [4,4096,128] f32."""
    from concourse.bass_utils import run_bass_kernel_spmd

    X = np.ascontiguousarray(X, dtype=np.float32)
    wmap = {"WQ": _prep_w(np.asarray(Wq, dtype=np.float32)),
            "WK": _prep_w(np.asarray(Wk, dtype=np.float32)),
            "WV": _prep_w(np.asarray(Wv, dtype=np.float32))}

    nc = _get_nc()
    in_maps = []
    for core in range(NCORES):
        b, half = core // QSPLIT, core % QSPLIT
        xb = X[b]
        if half:
            # roll so this core's queries are rows [0:NQ); key set is unchanged
            xb = np.concatenate([xb[NQ:], xb[:NQ]], axis=0)
        in_maps.append({"XT": _prep_xt(xb), **wmap})

    res = run_bass_kernel_spmd(
        nc, in_maps, core_ids=list(range(NCORES)), trace=trace
    )
    out = np.empty((B, N, H), dtype=np.float32)
    for core in range(NCORES):
        b, half = core // QSPLIT, core % QSPLIT
        out[b, half * NQ:(half + 1) * NQ] = res.results[core]["OT"].T
    if trace:
        return out, res
    return out
